# revision 17
# baseline (speedup 1.0000x reference)
"""Self-contained Trainium2 Bass kernel for nn_A3TGCNNet (A3TGCN GNN) — v3.

kernel(**inputs) -> np.ndarray [20000, 12]

v3 changes vs v2 (515us):
- host bin-packs each core's dsts into 19 groups of 128 + one of 68 so
  per-(group,window) gather cells flatten to <=32 edges: slot count
  drops ~153k -> ~100k (less ldweights, less oh DMA, fewer chunks)
- scatter one-hots precomputed on host as fp8 [128,128] stationaries
  (DMA instead of the 179us of DVE is_equal generation), 128-wide for
  fast weight load
- gate elementwise ops split across DVE and GpSimd by chunk parity;
  probs replicated into a flat [128,GC] tile (no 3D broadcast APs)
- degree reduction moved to GpSimd, w_win table in bf16 (half the DMA)
"""
import sys
sys.path.insert(0, "/opt/trn_rl_repo")

import math
import numpy as np
import ml_dtypes

import concourse.bass as bass
import concourse.bacc as bacc
import concourse.mybir as mybir
from concourse import tile

F32 = mybir.dt.float32
BF16 = mybir.dt.bfloat16
FP8 = mybir.dt.float8e4
AF = mybir.ActivationFunctionType
ALU = mybir.AluOpType
NP_FP8 = ml_dtypes.float8_e4m3


def make_cfg(N, E, P, H, O, ncores=8):
    NS = N // ncores
    assert NS * ncores == N
    # clusters of 512 dsts (last ragged), each split into groups of <=128
    NCL = math.ceil(NS / 512)
    csizes = [512] * (NCL - 1) + [NS - 512 * (NCL - 1)]
    gsizes = []
    cgroups = []  # per cluster: list of global group ids
    for cs in csizes:
        ids = []
        r = cs
        while r > 0:
            ids.append(len(gsizes))
            gsizes.append(min(128, r))
            r -= min(128, r)
        cgroups.append(ids)
    ngroups = len(gsizes)
    NW = math.ceil(N / 128)
    GC = 384  # gate chunk cols (multiple of P)
    return dict(N=N, E=E, P=P, H=H, O=O, ncores=ncores, ngroups=ngroups,
                gsizes=gsizes, NS=NS, NW=NW, GC=GC,
                NCL=NCL, csizes=csizes, cgroups=cgroups)


def _pack_subs(caps):
    """caps: [NW] per-window slot capacity (multiples of 32, 0 = skip).
    Pack into 128-col chunks; each sub is (window, slot_start, m) with
    slot_start 32-aligned and m<=128 not crossing a 128 boundary."""
    subs = []
    cur = 0
    for w in range(len(caps)):
        m = int(caps[w])
        while m > 0:
            room = 128 - (cur % 128)
            take = min(m, room)
            subs.append((w, cur, take))
            cur += take
            m -= take
    nslot = ((cur + 127) // 128) * 128
    while cur < nslot:
        subs.append((0, cur, 32))
        cur += 32
    return subs, nslot


def _binpack_groups(cnt_dw, gsizes, cap):
    """cnt_dw: [ND, NW] per-dst window histogram. Assign dsts to groups
    (sizes gsizes) flattening per-(group,window) totals toward <=cap.
    Returns members: list of arrays of dst-local ids."""
    ND, NW = cnt_dw.shape
    ng = len(gsizes)
    G = np.zeros((ng, NW), np.int32)
    sizes = np.zeros(ng, np.int32)
    gsz = np.asarray(gsizes)
    members = [[] for _ in range(ng)]
    order = np.argsort(-cnt_dw.sum(axis=1), kind="stable")
    for d in order:
        v = cnt_dw[d]
        nz = np.nonzero(v)[0]
        open_g = sizes < gsz
        if nz.size == 0:
            g = int(np.argmax(gsz - sizes))
        else:
            cand = G[:, nz] + v[nz][None, :]
            over = np.maximum(cand - cap, 0).sum(axis=1)
            peak = cand.max(axis=1)
            score = over * 1000.0 + peak + 0.002 * sizes
            score[~open_g] = 1e18
            g = int(np.argmin(score))
        members[g].append(d)
        G[g, nz] += v[nz]
        sizes[g] += 1
    return [np.asarray(m, np.int64) for m in members]


def host_prep(cfg, x, edge_index, edge_weight):
    N, P = cfg["N"], cfg["P"]
    ncores, ngroups = cfg["ncores"], cfg["ngroups"]
    NS, NW, NCL = cfg["NS"], cfg["NW"], cfg["NCL"]
    gsizes, csizes, cgroups = cfg["gsizes"], cfg["csizes"], cfg["cgroups"]
    E = edge_weight.shape[0]

    src = np.asarray(edge_index[0], dtype=np.int64)
    dst = np.asarray(edge_index[1], dtype=np.int64)
    w = np.asarray(edge_weight, dtype=np.float32)
    win = src // 128

    core = dst // NS
    dloc = dst % NS

    # --- per-core bin-packing of dsts into clusters of 512
    members_all = []   # [ncores][ngroups] arrays of local dst ids
    cid_of = np.zeros((ncores, NS), np.int32)   # local dst -> cluster
    qid_of = np.zeros((ncores, NS), np.int32)   # local dst -> group-in-cluster
    drel_of = np.zeros((ncores, NS), np.int32)  # local dst -> idx in group
    flat_perm = np.zeros((ncores, NS), np.int64)
    for c in range(ncores):
        m = core == c
        cw = np.zeros((NS, NW), np.int32)
        np.add.at(cw, (dloc[m], win[m]), 1)
        cmem = _binpack_groups(cw, csizes, cap=128)
        gmem = [None] * ngroups
        off = 0
        for cl in range(NCL):
            mem = cmem[cl]
            for qi, g in enumerate(cgroups[cl]):
                sub = mem[qi * 128:qi * 128 + gsizes[g]]
                gmem[g] = sub
                cid_of[c, sub] = cl
                qid_of[c, sub] = qi
                drel_of[c, sub] = np.arange(len(sub))
                flat_perm[c, off:off + len(sub)] = sub
                off += len(sub)
        assert off == NS
        members_all.append(gmem)

    cl_e = cid_of[core, dloc]
    q_e = qid_of[core, dloc]
    dr_e = drel_of[core, dloc]

    order = np.lexsort((src, win, cl_e, core))
    ss, ws_, wins, cls, cs = (src[order], w[order], win[order],
                              cl_e[order], core[order])
    qs_, drs = q_e[order], dr_e[order]
    NCC = ncores * NCL
    key = cs * NCL + cls
    gseg = np.searchsorted(key, np.arange(NCC + 1))

    cellcnt = np.zeros((ncores, NCL, NW), np.int64)
    cellstart = np.zeros((ncores, NCL, NW), np.int64)
    for c in range(ncores):
        for cl in range(NCL):
            gg = c * NCL + cl
            a, b = gseg[gg], gseg[gg + 1]
            wv = wins[a:b]
            st = np.searchsorted(wv, np.arange(NW + 1)) + a
            cellstart[c, cl] = st[:-1]
            cellcnt[c, cl] = st[1:] - st[:-1]

    caps = ((cellcnt.max(axis=0) + 31) // 32) * 32   # [NCL, NW]
    group_subs = []
    nslots = []
    for cl in range(NCL):
        subs, nslot = _pack_subs(caps[cl])
        group_subs.append(subs)
        nslots.append(nslot)
    NSLOTMAX = max(nslots)
    NCHMAX = NSLOTMAX // 128

    oh = np.zeros((ncores, NCL, 128, NSLOTMAX), NP_FP8)
    rseg = np.zeros((ncores, NCL, 128, NCHMAX, 128), NP_FP8)
    w_slot4 = np.zeros((ncores, NCL, 128, NCHMAX, 4), np.float32)

    for c in range(ncores):
        for cl in range(NCL):
            consumed = np.zeros(NW, np.int64)
            scol, srow, sdr, sq, swt = [], [], [], [], []
            for (wi, s0, m) in group_subs[cl]:
                have = cellcnt[c, cl, wi] - consumed[wi]
                take = int(max(0, min(m, have)))
                if take > 0:
                    e0 = cellstart[c, cl, wi] + consumed[wi]
                    scol.append(s0 + np.arange(take))
                    srow.append(ss[e0:e0 + take] % 128)
                    sdr.append(drs[e0:e0 + take])
                    sq.append(qs_[e0:e0 + take])
                    swt.append(ws_[e0:e0 + take])
                    consumed[wi] += take
            cols = np.concatenate(scol)
            rows = np.concatenate(srow)
            drv = np.concatenate(sdr)
            qv = np.concatenate(sq)
            wtv = np.concatenate(swt)
            oh[c, cl, rows, cols] = 1.0
            rseg[c, cl, cols % 128, cols // 128, drv] = 1.0
            w_slot4[c, cl, cols % 128, cols // 128, qv] = wtv

    # indegree weight table (global), replicated per core in window layout;
    # last column is the constant 1.0 self-loop weight
    indeg = np.bincount(dst, minlength=N)
    Lmax = max(2, int(indeg.max())) + 1
    order2 = np.argsort(dst, kind="stable")
    ds2, ws2 = dst[order2], w[order2]
    starts = np.searchsorted(ds2, np.arange(N), side="left")
    rank = np.arange(E) - starts[ds2]
    W = np.zeros((N, Lmax), np.float32)
    W[ds2, rank] = ws2
    W[:, -1] = 1.0

    Wfull = np.zeros((NW * 128, Lmax), np.float32)
    Wfull[:N] = W
    Wfull[:, -1] = 1.0
    w_win = np.ascontiguousarray(
        Wfull.reshape(NW, 128, Lmax).transpose(1, 0, 2)).astype(ml_dtypes.bfloat16)

    # own-dst tables in (group, idx) layout, padded to 128 rows
    didx = np.zeros((ncores, 128, ngroups), np.int64)
    valid = np.zeros((ncores, 128, ngroups), bool)
    for c in range(ncores):
        for g in range(ngroups):
            mem = members_all[c][g]
            didx[c, :len(mem), g] = c * NS + mem
            valid[c, :len(mem), g] = True
    w_pad = np.where(valid[..., None], W[didx], 0.0)
    w_pad[..., -1] = 1.0
    x_own = np.where(valid[..., None], np.asarray(x, np.float32)[didx], 0.0)

    xf = np.zeros((NW * 128, P), np.float32)
    xf[:N] = np.asarray(x, np.float32)
    x_win = np.ascontiguousarray(
        xf.reshape(NW, 128, P).transpose(1, 0, 2)).astype(ml_dtypes.bfloat16)

    meta = dict(NSLOTMAX=NSLOTMAX, NCHMAX=NCHMAX, Lmax=Lmax,
                group_subs=group_subs, nslots=nslots)
    tables = dict(oh=oh, rseg=rseg, w_slot4=w_slot4,
                  w_pad=w_pad.astype(np.float32),
                  x_own=x_own.astype(np.float32),
                  w_win=w_win, x_win=x_win, flat_perm=flat_perm)
    return meta, tables


def build(cfg, meta, debug=False):
    N, P, H, O = cfg["N"], cfg["P"], cfg["H"], cfg["O"]
    ncores, ngroups = cfg["ncores"], cfg["ngroups"]
    NS, NW, GC = cfg["NS"], cfg["NW"], cfg["GC"]
    gsizes = cfg["gsizes"]
    NCL, cgroups = cfg["NCL"], cfg["cgroups"]
    NSLOTMAX, NCHMAX, Lmax = meta["NSLOTMAX"], meta["NCHMAX"], meta["Lmax"]
    group_subs, nslots = meta["group_subs"], meta["nslots"]
    FMAX = max(gsizes) * P

    nc = bacc.Bacc(None, target_bir_lowering=False, debug=debug)

    x_win_p = nc.declare_dram_parameter("x_win", [128, NW, P], BF16, isOutput=False)
    w_win_p = nc.declare_dram_parameter("w_win", [128, NW, Lmax], BF16, isOutput=False)
    x_own = nc.declare_dram_parameter("x_own", [128, ngroups, P], F32, isOutput=False)
    w_pad = nc.declare_dram_parameter("w_pad", [128, ngroups, Lmax], F32, isOutput=False)
    oh_p = nc.declare_dram_parameter("oh", [NCL, 128, NSLOTMAX], FP8, isOutput=False)
    rseg_p = nc.declare_dram_parameter("rseg", [NCL, 128, NCHMAX, 128], FP8, isOutput=False)
    wslot_p = nc.declare_dram_parameter("w_slot4", [NCL, 128, NCHMAX, 4], F32, isOutput=False)
    att = nc.declare_dram_parameter("attention", [P], F32, isOutput=False)
    cwz = nc.declare_dram_parameter("conv_w_z", [1, H], F32, isOutput=False)
    cbz = nc.declare_dram_parameter("conv_b_z", [H], F32, isOutput=False)
    lwz = nc.declare_dram_parameter("lin_w_z", [2 * H, H], F32, isOutput=False)
    lbz = nc.declare_dram_parameter("lin_b_z", [H], F32, isOutput=False)
    cwh = nc.declare_dram_parameter("conv_w_h", [1, H], F32, isOutput=False)
    cbh = nc.declare_dram_parameter("conv_b_h", [H], F32, isOutput=False)
    lwh = nc.declare_dram_parameter("lin_w_h", [2 * H, H], F32, isOutput=False)
    lbh = nc.declare_dram_parameter("lin_b_h", [H], F32, isOutput=False)
    low = nc.declare_dram_parameter("lin_out_w", [H, O], F32, isOutput=False)
    lob = nc.declare_dram_parameter("lin_out_b", [O], F32, isOutput=False)
    out_ext = nc.declare_dram_parameter("out", [O, NS], F32, isOutput=True)

    agg_d = [nc.dram_tensor(f"agg_d{g}", [gsizes[g] * P], BF16) for g in range(ngroups)]

    with tile.TileContext(nc) as tc:
        with (
            tc.tile_pool(name="const", bufs=1) as cp,
            tc.tile_pool(name="ph1", bufs=1) as p1,
            tc.tile_pool(name="grp", bufs=2) as gp,
            tc.tile_pool(name="vsbp", bufs=2) as vp,
            tc.tile_pool(name="growp", bufs=8) as grp_,
            tc.tile_pool(name="gate", bufs=3) as tp,
            tc.tile_pool(name="ps_small", bufs=1, space="PSUM") as ps_s,
            tc.tile_pool(name="ps_val", bufs=2, space="PSUM") as ps_v,
            tc.tile_pool(name="ps_agg", bufs=1, space="PSUM") as ps_a,
            tc.tile_pool(name="ps_rep", bufs=2, space="PSUM") as ps_r,
        ):
            # ---- phase I bulk DMAs first so nothing queues ahead of them
            xs = p1.tile([128, NW, P], BF16)
            nc.sync.dma_start(xs[:], x_win_p.ap())
            CW = 40
            nck = math.ceil(NW / CW)
            wchs = []
            for k in range(nck):
                a, b = k * CW, min(NW, (k + 1) * CW)
                wch = p1.tile([128, CW, Lmax], BF16, tag=f"wch{k}")
                nc.sync.dma_start(wch[:, :b - a, :], w_win_p.ap()[:, a:b, :])
                wchs.append(wch)
            wp = p1.tile([128, ngroups, Lmax], F32)
            nc.sync.dma_start(wp[:], w_pad.ap())
            x_own_sb = p1.tile([128, ngroups, P], F32)
            nc.sync.dma_start(x_own_sb[:], x_own.ap())

            # ---- pipelined dinv + y per window chunk
            deg_win = p1.tile([128, NW], F32)
            dinv_win = p1.tile([128, NW], F32)
            y_sb = cp.tile([128, NW, P], BF16)
            for k in range(nck):
                a, b = k * CW, min(NW, (k + 1) * CW)
                nc.vector.tensor_reduce(deg_win[:, a:b], wchs[k][:, :b - a, :],
                                        axis=mybir.AxisListType.X, op=ALU.add)
                nc.scalar.activation(deg_win[:, a:b], deg_win[:, a:b], AF.Sqrt)
                nc.vector.reciprocal(dinv_win[:, a:b], deg_win[:, a:b])
                nc.vector.tensor_tensor(
                    y_sb[:, a:b, :], xs[:, a:b, :],
                    dinv_win[:, a:b].unsqueeze(-1).broadcast_to([128, b - a, P]),
                    op=ALU.mult)

            # own-dst dinv
            deg = p1.tile([128, ngroups], F32)
            nc.vector.tensor_reduce(deg[:], wp[:], axis=mybir.AxisListType.X, op=ALU.add)
            nc.scalar.activation(deg[:], deg[:], AF.Sqrt)
            dinv = p1.tile([128, ngroups], F32)
            nc.vector.reciprocal(dinv[:], deg[:])
            dinv2 = p1.tile([128, ngroups], F32)
            nc.vector.tensor_tensor(dinv2[:], dinv[:], dinv[:], op=ALU.mult)
            own_pre = p1.tile([128, ngroups, P], F32)
            nc.vector.tensor_tensor(
                own_pre[:], x_own_sb[:],
                dinv2[:].unsqueeze(-1).broadcast_to([128, ngroups, P]),
                op=ALU.mult)

            # ---- constants / gate affine params
            ones1f = cp.tile([1, 128], F32)
            nc.vector.memset(ones1f[:], 1.0)
            ones1 = cp.tile([1, 128], BF16)
            nc.vector.memset(ones1[:], 1.0)

            def gate_uv(lw, cw, cb, lb, negate):
                Wsb = cp.tile([H, H], F32, tag="Wsb")
                nc.sync.dma_start(Wsb[:], lw.ap()[0:H, :])
                cwc = cp.tile([H, 1], F32, tag="cwc")
                nc.sync.dma_start(cwc[:], cw.ap().rearrange("o k -> k o"))
                cbc = cp.tile([H, 1], F32, tag="cbc")
                nc.sync.dma_start(cbc[:], cb.ap().rearrange("(k o) -> k o", o=1))
                lbc = cp.tile([H, 1], F32, tag="lbc")
                nc.sync.dma_start(lbc[:], lb.ap().rearrange("(k o) -> k o", o=1))
                ups = ps_s.tile([H, 1], F32, tag="small_ps")
                nc.tensor.matmul(ups[:], Wsb[:], cwc[:], start=True, stop=True)
                u = cp.tile([H, 1], F32, tag=f"u{negate}")
                nc.vector.tensor_scalar_mul(u[:], ups[:], -1.0 if negate else 1.0)
                vps = ps_s.tile([H, 1], F32, tag="small_ps")
                nc.tensor.matmul(vps[:], Wsb[:], cbc[:], start=True, stop=True)
                v = cp.tile([H, 1], F32, tag=f"v{negate}")
                nc.vector.tensor_tensor(v[:], vps[:], lbc[:], op=ALU.add)
                if negate:
                    nc.vector.tensor_scalar_mul(v[:], v[:], -1.0)
                return u, v

            nuz, nvz = gate_uv(lwz, cwz, cbz, lbz, negate=True)
            uh, vh = gate_uv(lwh, cwh, cbh, lbh, negate=False)

            atts = cp.tile([1, P], F32)
            nc.sync.dma_start(atts[:], att.ap().rearrange("(o p) -> o p", o=1))
            pex = cp.tile([1, P], F32)
            nc.scalar.activation(pex[:], atts[:], AF.Exp)
            psum_t = cp.tile([1, 1], F32)
            nc.vector.tensor_reduce(psum_t[:], pex[:], axis=mybir.AxisListType.X, op=ALU.add)
            prcp = cp.tile([1, 1], F32)
            nc.vector.reciprocal(prcp[:], psum_t[:])
            probs1 = cp.tile([1, P], F32)
            nc.vector.tensor_scalar(probs1[:], pex[:], prcp[:, 0:1], None, op0=ALU.mult)
            prps = ps_s.tile([128, P], F32, tag="small_ps")
            nc.tensor.matmul(prps[:], ones1f[:], probs1[:], start=True, stop=True)
            probs_bf = cp.tile([128, P], BF16)
            nc.vector.tensor_copy(probs_bf[:], prps[:])
            probs_t = cp.tile([128, GC], BF16)
            nc.vector.tensor_copy(
                probs_t[:].rearrange("k (n p) -> k n p", p=P),
                probs_bf[:].unsqueeze(1).broadcast_to([128, GC // P, P]))

            lows_f = cp.tile([H, O], F32)
            nc.sync.dma_start(lows_f[:], low.ap())
            lows = cp.tile([H, O], BF16)
            nc.vector.tensor_copy(lows[:], lows_f[:])
            lobc = cp.tile([O, 1], F32)
            nc.sync.dma_start(lobc[:], lob.ap().rearrange("(o i) -> o i", i=1))

            h_all = cp.tile([128, NS], F32)

            # ---- phase II: quarter-interleaved gather/scatter pipeline,
            # gates one cluster behind, spread one group per quarter
            QS = 42

            def load_quarter(cl, qr):
                nch = nslots[cl] // 128
                c0, c1 = qr * QS, min(nch, (qr + 1) * QS)
                if c0 >= nch:
                    return
                ohg, rsg, wsg4 = pending[cl]
                nc.sync.dma_start(ohg[:, c0 * 128:c1 * 128],
                                  oh_p.ap()[cl, :, c0 * 128:c1 * 128])
                nc.sync.dma_start(rsg[:, c0:c1, :], rseg_p.ap()[cl, :, c0:c1, :])
                nc.sync.dma_start(wsg4[:, c0:c1, :], wslot_p.ap()[cl, :, c0:c1, :])

            def alloc_tables(cl):
                ohg = gp.tile([128, NSLOTMAX], FP8, tag="ohg", name=f"ohg{cl}")
                rsg = gp.tile([128, NCHMAX, 128], FP8, tag="rsg", name=f"rsg{cl}")
                wsg4 = gp.tile([128, NCHMAX, 4], F32, tag="wsg4", name=f"wsg4{cl}")
                return ohg, rsg, wsg4

            pending = {0: alloc_tables(0)}
            for qr in range(4):
                load_quarter(0, qr)

            def gates_out(g, grow):
                F = gsizes[g] * P
                cb = 128 * g
                nchk = math.ceil(F / GC)
                for k in range(nchk):
                    a, b = k * GC, min(F, (k + 1) * GC)
                    rep = ps_r.tile([128, 512], F32, tag="rep", name="rep")
                    nc.tensor.matmul(rep[:, :b - a], ones1[:], grow[:1, a:b],
                                     start=True, stop=True)
                    omz = tp.tile([128, GC], BF16, tag="omz")
                    nc.scalar.activation(omz[:, :b - a], rep[:, :b - a], AF.Sigmoid,
                                         scale=nuz[:, 0:1], bias=nvz[:, 0:1])
                    th = tp.tile([128, GC], BF16, tag="th")
                    nc.scalar.activation(th[:, :b - a], rep[:, :b - a], AF.Tanh,
                                         scale=uh[:, 0:1], bias=vh[:, 0:1])
                    e1 = nc.gpsimd if (g + k) % 2 == 0 else nc.vector
                    e2 = nc.vector if (g + k) % 2 == 0 else nc.gpsimd
                    e1.tensor_tensor(th[:, :b - a], th[:, :b - a],
                                     probs_t[:, :b - a], op=ALU.mult)
                    e2.tensor_tensor(omz[:, :b - a], omz[:, :b - a],
                                     th[:, :b - a], op=ALU.mult)
                    nc.vector.tensor_reduce(
                        h_all[:, cb + a // P: cb + b // P],
                        omz[:, :b - a].rearrange("k (n p) -> k n p", p=P),
                        axis=mybir.AxisListType.X, op=ALU.add)

            def epilogue(cl):
                c0 = 512 * cl
                cw_ = min(NS - c0, 512)
                hs = h_all[:, c0:c0 + cw_]
                mneg = tp.tile([128, 512], F32, tag="mneg")
                nc.vector.tensor_scalar_min(mneg[:, :cw_], hs, 0.0)
                nc.scalar.activation(mneg[:, :cw_], mneg[:, :cw_], AF.Exp)
                eluh = tp.tile([128, 512], F32, tag="eluh")
                nc.vector.tensor_scalar_max(eluh[:, :cw_], hs, 0.0)
                nc.vector.tensor_tensor(eluh[:, :cw_], eluh[:, :cw_], mneg[:, :cw_],
                                        op=ALU.add)
                eluhb = tp.tile([128, 512], BF16, tag="eluhb")
                nc.vector.tensor_scalar_add(eluhb[:, :cw_], eluh[:, :cw_], -1.0)
                ops = ps_r.tile([128, 512], F32, tag="rep", name=f"ops{cl}")
                nc.tensor.matmul(ops[:O, :cw_], lows[:], eluhb[:, :cw_],
                                 start=True, stop=True)
                osb = tp.tile([O, 512], F32, tag="osb")
                nc.vector.tensor_scalar(osb[:, :cw_], ops[:O, :cw_],
                                        lobc[:, 0:1], None, op0=ALU.add)
                nc.sync.dma_start(out_ext.ap()[:, c0:c0 + cw_], osb[:, :cw_])

            grows = {}
            for cl in range(NCL):
                gids = cgroups[cl]
                ngq = len(gids)
                g0 = gids[0]
                nslot = nslots[cl]
                nch = nslot // 128
                nq = math.ceil(nch / QS)
                ohg, rsg, wsg4 = pending[cl]
                subs = group_subs[cl]
                if cl + 1 < NCL:
                    pending[cl + 1] = alloc_tables(cl + 1)
                if cl >= 1:
                    for gp_ in cgroups[cl - 1]:
                        gates_out(gp_, grows.pop(gp_))
                if cl >= 2:
                    epilogue(cl - 2)

                NAB = 3
                aggps = [ps_a.tile([128, 48], F32, tag=f"agg{b}", name=f"agg{b}")
                         for b in range(NAB)]
                for qr in range(nq):
                    c0, c1 = qr * QS, min(nch, (qr + 1) * QS)
                    vbank = ps_v.tile([128, 504], F32, tag="vb", name=f"vb{cl}_{qr}")
                    for (wi, s0, m) in subs:
                        ch = s0 // 128
                        if not (c0 <= ch < c1):
                            continue
                        pcol = (ch - c0) * P
                        nc.tensor.matmul(
                            vbank[(s0 % 128):(s0 % 128) + m, pcol:pcol + P],
                            ohg[:, s0:s0 + m], y_sb[:, wi, :],
                            start=True, stop=True, tile_position=(0, s0 % 128))
                    vsb4 = vp.tile([128, QS * 48], BF16, tag=f"vsb{qr % 2}",
                                   name=f"vsb{cl}_{qr}")
                    nco = c1 - c0
                    nc.vector.tensor_tensor(
                        vsb4[:, :nco * 48].rearrange("e (c q j) -> e c q j", q=4, j=P),
                        vbank[:, :nco * P].rearrange("e (c j) -> e c j", j=P)
                        .unsqueeze(2).broadcast_to([128, nco, 4, P]),
                        wsg4[:, c0:c1, :].unsqueeze(-1).broadcast_to([128, nco, 4, P]),
                        op=ALU.mult)
                    for ch in range(c0, c1):
                        nc.tensor.matmul(aggps[ch % NAB][:, :48], rsg[:, ch, :],
                                         vsb4[:, (ch - c0) * 48:(ch - c0 + 1) * 48],
                                         start=(ch < NAB), stop=(ch >= nch - NAB))
                    if cl + 1 < NCL:
                        load_quarter(cl + 1, qr)

                agg_sb = gp.tile([128, 48], F32, tag="agg_sb")
                nc.vector.tensor_copy(agg_sb[:], aggps[0][:])
                nc.vector.tensor_tensor(agg_sb[:], agg_sb[:], aggps[1][:], op=ALU.add)
                nc.vector.tensor_tensor(agg_sb[:], agg_sb[:], aggps[2][:], op=ALU.add)

                aggbf = gp.tile([128, 48], BF16, tag="aggbf")
                nc.vector.tensor_tensor(
                    aggbf[:].rearrange("e (q j) -> e q j", j=P),
                    agg_sb[:].rearrange("e (q j) -> e q j", j=P),
                    dinv[:, g0:g0 + ngq].unsqueeze(-1).broadcast_to([128, ngq, P]),
                    op=ALU.mult)
                nc.vector.tensor_tensor(
                    aggbf[:].rearrange("e (q j) -> e q j", j=P),
                    aggbf[:].rearrange("e (q j) -> e q j", j=P),
                    own_pre[:, g0:g0 + ngq, :],
                    op=ALU.add)

                for qi, g in enumerate(gids):
                    GN = gsizes[g]
                    nc.sync.dma_start(
                        agg_d[g].ap().rearrange("(d p) -> d p", p=P),
                        aggbf[:GN, qi * P:(qi + 1) * P])
                    grow = grp_.tile([1, FMAX], BF16, tag="grow", name=f"grow{g}")
                    nc.sync.dma_start(
                        grow[:, :GN * P],
                        agg_d[g].ap().rearrange("(o f) -> o f", o=1))
                    grows[g] = grow

            for g in cgroups[NCL - 1]:
                gates_out(g, grows.pop(g))
            epilogue(NCL - 2)
            epilogue(NCL - 1)

    nc.compile()
    return nc


def assemble(cfg, results, flat_perm):
    N, O, NS = cfg["N"], cfg["O"], cfg["NS"]
    out = np.zeros((N, O), np.float32)
    for c in range(cfg["ncores"]):
        oc = np.asarray(results[c]["out"])  # [O, NS]
        out[c * NS + flat_perm[c]] = oc.T
    return out


def make_inmaps(cfg, inputs, tables):
    keys = ["attention", "conv_w_z", "conv_b_z", "lin_w_z", "lin_b_z",
            "conv_w_h", "conv_b_h", "lin_w_h", "lin_b_h", "lin_out_w", "lin_out_b"]
    in_maps = []
    for c in range(cfg["ncores"]):
        m = {k: np.ascontiguousarray(inputs[k], np.float32) for k in keys}
        m["x_win"] = tables["x_win"]
        m["w_win"] = tables["w_win"]
        m["x_own"] = tables["x_own"][c]
        m["w_pad"] = tables["w_pad"][c]
        m["oh"] = tables["oh"][c]
        m["rseg"] = tables["rseg"][c]
        m["w_slot4"] = tables["w_slot4"][c]
        in_maps.append(m)
    return in_maps


_CACHE = {}


def kernel(**inputs):
    import numpy as _np
    from concourse import bass_utils as _bu
    x = _np.asarray(inputs["x"], _np.float32)
    ei = _np.asarray(inputs["edge_index"])
    ew = _np.asarray(inputs["edge_weight"], _np.float32)
    N, P = x.shape
    E = ew.shape[0]
    H = _np.asarray(inputs["lin_b_z"]).shape[0]
    O = _np.asarray(inputs["lin_out_b"]).shape[0]
    cfg = make_cfg(N, E, P, H, O, ncores=8)
    meta, tables = host_prep(cfg, x, ei, ew)
    key = (N, E, P, H, O, meta["NSLOTMAX"], meta["NCHMAX"], meta["Lmax"],
           tuple(meta["nslots"]),
           tuple(tuple(s) for subs in meta["group_subs"] for s in subs))
    if key in _CACHE:
        nc = _CACHE[key]
    else:
        nc = build(cfg, meta, debug=False)
        _CACHE[key] = nc
    in_maps = make_inmaps(cfg, inputs, tables)
    res = _bu.run_bass_kernel_spmd(nc, in_maps, core_ids=list(range(8)))
    return assemble(cfg, res.results, tables["flat_perm"])
